# revision 27
# baseline (speedup 1.0000x reference)
"""AttentiveFP (3x GAT + segment-softmax GRU readout) on 8 Trainium2 cores.

Sharding: data-parallel over graphs. Core c owns graphs [128c, 128(c+1)) and
their nodes (batch is sorted). Per GAT layer each core aggregates messages for
its own destination nodes (dst-major degree-bucketed grid), then the per-node
feature table (h | exp(a_src) | exp(0.2 a_src), fp32) is AllGathered so every
core can gather arbitrary source rows next layer. The GAT aggregation is done
on h directly (sum_e alpha_e h[src] then @W afterwards - linearity), and
exp(leaky(a_s+a_d)) is computed separably as max(e^as * e^ad, e^.2as * e^.2ad)
so edge scores need no per-edge cross terms. Readout: graph-major grid,
softmax + context are per-partition ops; GRU runs twice on a loop-invariant
context; output MLP on [128 graphs, H] per core.

Host->device traffic is minimized (the runs are PJRT-proxied over a slow
tunnel, so input bytes + per-call jit overhead dominate wall time, not device
compute): node features ship as fp8-e3m4 (~3.3MB), weights as one fp8 blob
scaled x16 into e3m4's normal range (un-scaled during the on-device upconvert
to bf16), gather/scatter indices as one uint16 blob (cast to int32 by a SWDGE
DMA on device), and only the exp-amplified attention row vectors stay f32.
Replicated per-partition weight tiles are built on device with a ones-column
matmul instead of being shipped 128x. The XLA executable is cached across
calls via the jax persistent compilation cache, and the BIR JSON serialization
is memoized on the Bass instance (it is re-encoded inside every jit lowering
otherwise). Per-window gathers stay one-indirect-DMA-per-degree-slot: the HW
ucode only honors a single offset column per partition (batched multi-column
offset APs silently misaddress), and dma_gather needs int16 row ids which a
52k-row table cannot satisfy.
"""

import numpy as np
import ml_dtypes

try:  # cache the XLA executable across run_bass_kernel_spmd calls
    import jax as _jax

    _jax.config.update("jax_compilation_cache_dir", "/tmp/jax_comp_cache")
    _jax.config.update("jax_persistent_cache_min_entry_size_bytes", -1)
    _jax.config.update("jax_persistent_cache_min_compile_time_secs", 0.0)
except Exception:
    pass

BF16 = np.dtype(ml_dtypes.bfloat16)
F8 = np.dtype(ml_dtypes.float8_e3m4)

H = 128
L = 3
P = 128
NCORES = 8
TC = 132  # table row: h(128) | exp(a_s) | exp(.2 a_s) | pad | pad


def _wb_layout(meta):
    """fp8 (e3m4) blob layout: name -> (elem offset, rows, cols)."""
    segs = [("encW", 64, H)]
    for k in range(L):
        segs.append((f"gatW{k}", H, H))
    segs += [("attW1", H, H), ("WihT", H, 3 * H), ("WhhT", H, 3 * H),
             ("W1o", H, H), ("W2o", H, 64), ("W3o", 64, 1), ("encb", 1, H)]
    for k in range(L):
        segs.append((f"gatb{k}", 1, H))
    segs += [("attb1", 1, H), ("w2", 1, H), ("bih", 1, 3 * H), ("bhh", 1, 3 * H),
             ("b1o", 1, H), ("b2o", 1, 64), ("b3o", 1, 1)]
    out, cur = {}, 0
    for name, r, c in segs:
        out[name] = (cur, r, c)
        cur += r * c
    cur = ((cur + NCORES - 1) // NCORES) * NCORES  # pad so the blob shards 8-way
    return out, cur


def _fb_layout():
    """f32 blob: attention score rows (exp-amplified, keep full precision)."""
    out, cur = {}, 0
    for k in range(L):
        out[f"ws{k}"] = (cur, 1, H)
        cur += H
    for k in range(L):
        out[f"wd{k}"] = (cur, 1, H)
        cur += H
    return out, cur


def _build_host(x, edge_index, batch):
    N = x.shape[0]
    G = 1024
    gpc = G // NCORES

    src = np.concatenate([np.arange(N, dtype=np.int64), edge_index[0].astype(np.int64)])
    dst = np.concatenate([np.arange(N, dtype=np.int64), edge_index[1].astype(np.int64)])

    node_core = (batch // gpc).astype(np.int64)
    S = np.bincount(node_core, minlength=NCORES)
    indeg = np.bincount(dst, minlength=N)

    # order nodes within each core by in-degree descending
    node_pos = np.zeros(N, np.int64)
    perm = []
    for c in range(NCORES):
        cn = np.where(node_core == c)[0]
        order = cn[np.argsort(-indeg[cn], kind="stable")]
        perm.append(order)
        node_pos[order] = np.arange(len(order))

    NW = int(max((s + P - 1) // P for s in S))
    S_pad = (NW + 1) * P
    R_tot = NCORES * S_pad
    sent_row = np.array([c * S_pad + NW * P for c in range(NCORES)], np.int64)
    table_row = node_core * S_pad + node_pos

    # per-window max in-window degree, equalized across cores
    win_of_node = node_pos // P
    Tw = np.zeros(NW, np.int64)
    np.maximum.at(Tw, win_of_node, indeg)
    Tw = np.maximum(Tw, 1)
    SUMT = int(Tw.sum())
    t_off = np.concatenate([[0], np.cumsum(Tw)]).astype(np.int64)

    # j-rank of each edge within its dst (vectorized)
    order_e = np.argsort(dst, kind="stable")
    ds = dst[order_e]
    starts = np.concatenate([[0], np.cumsum(np.bincount(ds, minlength=N))[:-1]])
    jrank = np.arange(len(ds)) - starts[ds]

    e_core = node_core[ds]
    dloc = node_pos[ds]
    col = t_off[dloc // P] + jrank
    row = dloc % P
    assert R_tot < 65536
    offs = np.empty((NCORES, P, SUMT), np.uint16)
    for c in range(NCORES):
        offs[c, :, :] = sent_row[c]
        sel = e_core == c
        offs[c, row[sel], col[sel]] = table_row[src[order_e][sel]]

    # readout grid
    gsizes = np.bincount(batch, minlength=G)
    J = int(gsizes.max())
    gstarts = np.concatenate([[0], np.cumsum(gsizes)[:-1]])
    j_in_graph = np.arange(N) - gstarts[batch]
    g_loc = (batch - node_core * gpc).astype(np.int64)
    grid_rows = gpc * J + 1
    junk_row = gpc * J
    assert grid_rows < 65536
    scat = np.full((NCORES, P, NW), junk_row, np.uint16)
    for c in range(NCORES):
        idx = np.arange(len(perm[c]))
        scat[c, idx % P, idx // P] = g_loc[perm[c]] * J + j_in_graph[perm[c]]

    xT = np.zeros((NCORES, x.shape[1], NW * P), F8)
    for c in range(NCORES):
        xT[c, :, :S[c]] = x[perm[c]].T.astype(F8)

    return dict(NW=NW, S_pad=S_pad, R_tot=R_tot, Tw=Tw, SUMT=SUMT,
                t_off=t_off, J=J, GR=grid_rows, gpc=gpc), offs, scat, xT


def _pack_weights(enc_W, enc_b, gat_W, gat_a_src, gat_a_dst, gat_b,
                  att_W1, att_b1, att_w2, att_b2, gru_Wih, gru_Whh,
                  gru_bih, gru_bhh, out_W1, out_b1, out_W2, out_b2,
                  out_W3, out_b3):
    f = lambda a: np.asarray(a, np.float32)
    w = dict(
        encW=f(enc_W), encb=f(enc_b),
        attW1=f(att_W1), attb1=f(att_b1), w2=f(att_w2),
        WihT=np.ascontiguousarray(f(gru_Wih).T),
        WhhT=np.ascontiguousarray(f(gru_Whh).T),
        bih=f(gru_bih), bhh=f(gru_bhh),
        W1o=f(out_W1), b1o=f(out_b1),
        W2o=f(out_W2), b2o=f(out_b2),
        W3o=f(out_W3), b3o=f(out_b3).reshape(1),
    )
    for k in range(L):
        w[f"gatW{k}"] = f(gat_W[k])
        w[f"gatb{k}"] = f(gat_b[k])
        w[f"ws{k}"] = f(gat_W[k]) @ f(gat_a_src[k])
        w[f"wd{k}"] = f(gat_W[k]) @ f(gat_a_dst[k])
    return w


def _make_inmaps(meta, offs, scat, xT, weights):
    woff, nbw = _wb_layout(meta)
    foff, nbf = _fb_layout()
    SUMT, NW = meta["SUMT"], meta["NW"]

    # weights are ~0.08-scale: x16 keeps them in e3m4's normal range (the
    # device multiplies by 1/16 during the fp8->bf16 upconvert)
    wb = np.zeros(nbw, F8)
    for name, (a, r, c) in woff.items():
        wb[a:a + r * c] = (weights[name] * 16.0).astype(F8).reshape(-1)
    fb = np.zeros(nbf, np.float32)
    for name, (a, r, c) in foff.items():
        fb[a:a + r * c] = weights[name].reshape(-1)

    # each core ships 1/8th of the (identical) weight blob; the kernel
    # AllGathers the full blob on device before unpacking
    s8 = nbw // NCORES
    in_maps = []
    for c in range(NCORES):
        ib = np.concatenate([offs[c].reshape(-1), scat[c].reshape(-1)])
        in_maps.append(dict(wb=wb[c * s8:(c + 1) * s8], ib=ib, fb=fb,
                            xb=xT[c].reshape(-1)))
    return in_maps


def _build_bass(meta, stage=99):
    import concourse.bass as bass
    import concourse.mybir as mybir
    import concourse.tile as tile
    import concourse.bacc as bacc
    from concourse.tile import add_dep_helper
    from concourse.masks import make_identity

    f32 = mybir.dt.float32
    bf16 = mybir.dt.bfloat16
    f8 = mybir.dt.float8e3
    u16 = mybir.dt.uint16
    i32 = mybir.dt.int32
    AOP = mybir.AluOpType
    ACT = mybir.ActivationFunctionType
    NW, S_pad, R_tot = meta["NW"], meta["S_pad"], meta["R_tot"]
    Tw, t_off, J, GR = meta["Tw"], meta["t_off"], meta["J"], meta["GR"]
    SUMT = meta["SUMT"]

    woff, nbw = _wb_layout(meta)
    foff, nbf = _fb_layout()

    nc = bacc.Bacc("TRN2", target_bir_lowering=False, debug=False, num_devices=NCORES)

    s8 = nbw // NCORES
    wb_in = nc.dram_tensor("wb", [s8], f8, kind="ExternalInput")
    ib_in = nc.dram_tensor("ib", [P * SUMT + P * NW], u16, kind="ExternalInput")
    fb_in = nc.dram_tensor("fb", [nbf], f32, kind="ExternalInput")
    xb_in = nc.dram_tensor("xb", [64 * NW * P], f8, kind="ExternalInput")
    wb_stage = nc.dram_tensor("wb_stage", [s8], f8)
    wb_full = nc.dram_tensor("wb_full", [nbw], f8)
    out_t = nc.dram_tensor("out", [P, 1], f32, kind="ExternalOutput")

    own_tab = [nc.dram_tensor(f"own{k}", [S_pad, TC], f32) for k in range(L)]
    tables = [nc.dram_tensor(f"tab{k}", [R_tot, TC], f32) for k in range(L)]
    grid3 = nc.dram_tensor("grid3", [GR, TC], f32)

    def wview(name):
        a, r, c = woff[name]
        return wb_full[a:a + r * c].rearrange("(r c) -> r c", r=r, c=c)

    def fview(name):
        a, r, c = foff[name]
        return fb_in[a:a + r * c].rearrange("(r c) -> r c", r=r, c=c)

    with tile.TileContext(nc) as tc:
        with (
            tc.tile_pool(name="const", bufs=1) as cp,
            tc.tile_pool(name="sb", bufs=3) as sb,
            tc.tile_pool(name="gth", bufs=2) as gp,
            tc.tile_pool(name="mbuf", bufs=2) as mp,
            tc.tile_pool(name="ps", bufs=2, space="PSUM") as pp,
            tc.tile_pool(name="ps2", bufs=2, space="PSUM") as pp2,
        ):
            nc.sync.dma_start(out=wb_stage[:], in_=wb_in[:])
            nc.gpsimd.collective_compute(
                "AllGather", AOP.bypass, replica_groups=[list(range(NCORES))],
                ins=[wb_stage[:].opt()], outs=[wb_full[:].opt()])

            ident = cp.tile([P, P], f32)
            make_identity(nc, ident[:])
            ones_row = cp.tile([1, P], bf16)
            nc.vector.memset(ones_row[:], 1.0)
            ones_f32 = cp.tile([1, P], f32)
            nc.vector.memset(ones_f32[:], 1.0)

            _dc = [0]

            def wtile(name):
                a, r, c = woff[name]
                _dc[0] += 1
                t8 = cp.tile([r, c], f8, tag=f"c{_dc[0]}q", name=f"c{_dc[0]}q")
                nc.sync.dma_start(out=t8[:], in_=wview(name))
                t = cp.tile([r, c], bf16, tag=f"c{_dc[0]}", name=f"c{_dc[0]}")
                nc.vector.tensor_scalar(out=t[:], in0=t8[:], scalar1=0.0625,
                                        scalar2=None, op0=AOP.mult)
                return t

            def reprow(rowtile, n, dt):
                """[1,n] SBUF row tile -> [P,n] replicated f32 SBUF tile."""
                _dc[0] += 1
                ps = pp2.tile([P, 3 * H], f32, tag="ps2t", name="ps2r")
                nc.tensor.matmul(ps[:, 0:n],
                                 lhsT=(ones_f32 if dt == f32 else ones_row)[:],
                                 rhs=rowtile[:], start=True, stop=True)
                t = cp.tile([P, n], f32, tag=f"c{_dc[0]}r", name=f"c{_dc[0]}r")
                nc.vector.tensor_copy(out=t[:], in_=ps[:, 0:n])
                return t

            def frow(name):
                a, r, c = foff[name]
                _dc[0] += 1
                t = cp.tile([1, c], f32, tag=f"c{_dc[0]}", name=f"c{_dc[0]}")
                nc.sync.dma_start(out=t[:], in_=fview(name))
                return t

            offs_sb = cp.tile([P, SUMT], i32)
            offs_ld = nc.gpsimd.dma_start(
                out=offs_sb[:],
                in_=ib_in[0:P * SUMT].rearrange("(r c) -> r c", r=P, c=SUMT))
            scat_sb = cp.tile([P, NW], i32)
            scat_ld = nc.gpsimd.dma_start(
                out=scat_sb[:],
                in_=ib_in[P * SUMT:P * SUMT + P * NW].rearrange("(r c) -> r c", r=P, c=NW))

            x8 = cp.tile([64, NW * P], f8)
            nc.sync.dma_start(out=x8[:], in_=xb_in[:].rearrange("(r c) -> r c", r=64, c=NW * P))
            xall = cp.tile([64, NW * P], bf16)
            nc.vector.tensor_copy(out=xall[:], in_=x8[:])
            encW = wtile("encW")
            encb = wtile("encb")
            gatW = [wtile(f"gatW{k}") for k in range(L)]
            gatb = [wtile(f"gatb{k}") for k in range(L)]
            attW1 = wtile("attW1")
            attb1 = wtile("attb1")
            WihT = wtile("WihT")
            WhhT = wtile("WhhT")
            bih = wtile("bih")
            bhh = wtile("bhh")
            W1o = wtile("W1o")
            b1o = wtile("b1o")
            W2o = wtile("W2o")
            b2o = wtile("b2o")
            W3o = wtile("W3o")
            b3o = wtile("b3o")

            wsrep = [reprow(frow(f"ws{k}"), H, f32) for k in range(L)]
            wdrep = [reprow(frow(f"wd{k}"), H, f32) for k in range(L)]
            w2rep = reprow(wtile("w2"), H, bf16)
            bhhr = reprow(wtile("bhh"), 3 * H, bf16)

            Bbuf = [cp.tile([P, NW], f32, tag=f"B{k}", name=f"Bbuf{k}") for k in range(L)]
            B2buf = [cp.tile([P, NW], f32, tag=f"B2{k}", name=f"B2buf{k}") for k in range(L)]

            def score_cols(h_view, k, w, tile_t):
                tmp = sb.tile([P, H], f32, tag="ttrtmp")
                nc.vector.tensor_tensor(out=tmp[:], in0=h_view, in1=wsrep[k][:], op=AOP.mult)
                a_s = sb.tile([P, 1], f32, tag="as")
                nc.vector.tensor_reduce(out=a_s[:], in_=tmp[:], axis=mybir.AxisListType.X, op=AOP.add)
                nc.scalar.activation(tile_t[:, 128:129], a_s[:], ACT.Exp)
                nc.scalar.activation(tile_t[:, 129:130], a_s[:], ACT.Exp, scale=0.2)
                tmp2 = sb.tile([P, H], f32, tag="ttrtmp2")
                nc.vector.tensor_tensor(out=tmp2[:], in0=h_view, in1=wdrep[k][:], op=AOP.mult)
                a_d = sb.tile([P, 1], f32, tag="ad")
                nc.vector.tensor_reduce(out=a_d[:], in_=tmp2[:], axis=mybir.AxisListType.X, op=AOP.add)
                nc.scalar.activation(Bbuf[k][:, w:w + 1], a_d[:], ACT.Exp)
                nc.scalar.activation(B2buf[k][:, w:w + 1], a_d[:], ACT.Exp, scale=0.2)

            def write_sentinel(tab):
                z = sb.tile([P, TC], f32, tag="tab")
                nc.vector.memset(z[:], 0.0)
                nc.sync.dma_start(out=tab[NW * P:(NW + 1) * P, :], in_=z[:])

            # ---------------- encoder ----------------
            for w in range(NW):
                ps = pp.tile([P, P], f32, tag="pst", name="ppa")
                nc.tensor.matmul(ps[:], lhsT=xall[:, w * P:(w + 1) * P], rhs=encW[:],
                                 start=True, stop=False)
                nc.tensor.matmul(ps[:], lhsT=ones_row[:], rhs=encb[:], start=False, stop=True)
                tile_t = sb.tile([P, TC], f32, tag="tab")
                nc.scalar.activation(tile_t[:, 0:H], ps[:], ACT.Relu)
                nc.vector.memset(tile_t[:, 130:132], 0.0)
                score_cols(tile_t[:, 0:H], 0, w, tile_t)
                nc.sync.dma_start(out=own_tab[0][w * P:(w + 1) * P, :], in_=tile_t[:])
            write_sentinel(own_tab[0])

            if stage >= 1:
                nc.gpsimd.collective_compute(
                    "AllGather", AOP.bypass, replica_groups=[list(range(NCORES))],
                    ins=[own_tab[0][:, :].opt()], outs=[tables[0][:, :].opt()])

            if stage <= 1:
                dbg = sb.tile([P, 1], f32, tag="dbg")
                nc.sync.dma_start(out=dbg[:], in_=(tables[0] if stage >= 1 else own_tab[0])[0:P, 0:1])
                nc.sync.dma_start(out=out_t[:, :], in_=dbg[:])
            zg_t = cp.tile([P, TC], f32)
            nc.vector.memset(zg_t[:], 0.0)
            for b in range((GR + P - 1) // P):
                lo = b * P
                hi = min(GR, lo + P)
                nc.sync.dma_start(out=grid3[lo:hi, :], in_=zg_t[0:hi - lo, :])

            # ---------------- GAT layers ----------------
            for k in range(L if stage >= 3 else (1 if stage == 2 else 0)):
                for w in range(NW):
                    T = int(Tw[w])
                    Gw = gp.tile([P, T, TC], f32, tag="G", bufs=3)
                    nc.sync.dma_start(out=Gw[:, 0, :],
                                      in_=own_tab[k][w * P:(w + 1) * P, :])
                    for j in range(1, T):
                        gi = nc.gpsimd.indirect_dma_start(
                            out=Gw[:, j, :], out_offset=None,
                            in_=tables[k][:, :],
                            in_offset=bass.IndirectOffsetOnAxis(
                                ap=offs_sb[:, t_off[w] + j:t_off[w] + j + 1],
                                axis=0))
                        add_dep_helper(gi.ins, offs_ld.ins, reason="offs")
                    u1 = sb.tile([P, T], f32, tag="u1")
                    nc.vector.tensor_scalar(out=u1[:], in0=Gw[:, :, 128],
                                            scalar1=Bbuf[k][:, w:w + 1],
                                            scalar2=None, op0=AOP.mult)
                    u2 = sb.tile([P, T], f32, tag="u2")
                    nc.vector.tensor_scalar(out=u2[:], in0=Gw[:, :, 129],
                                            scalar1=B2buf[k][:, w:w + 1],
                                            scalar2=None, op0=AOP.mult)
                    wt = sb.tile([P, T], f32, tag="wt")
                    nc.vector.tensor_tensor(out=wt[:], in0=u1[:], in1=u2[:], op=AOP.max)
                    z = sb.tile([P, 1], f32, tag="z")
                    nc.vector.tensor_reduce(out=z[:], in_=wt[:],
                                            axis=mybir.AxisListType.X, op=AOP.add)
                    nc.vector.tensor_scalar(out=z[:], in0=z[:], scalar1=1e-16,
                                            scalar2=None, op0=AOP.add)
                    r = sb.tile([P, 1], f32, tag="r")
                    nc.vector.reciprocal(out=r[:], in_=z[:])
                    al = sb.tile([P, T], f32, tag="al")
                    nc.vector.tensor_scalar(out=al[:], in0=wt[:], scalar1=r[:],
                                            scalar2=None, op0=AOP.mult)
                    M = mp.tile([P, H * T], f32, tag="M", bufs=1)
                    nc.vector.tensor_tensor(
                        out=M[:].rearrange("p (h j) -> p j h", j=T, h=H),
                        in0=Gw[:, :, 0:H],
                        in1=al[:].to_broadcast([P, T, H]),
                        op=AOP.mult)
                    hagg = sb.tile([P, H], f32, tag="hagg")
                    nc.vector.tensor_reduce(
                        out=hagg[:], in_=M[:].rearrange("p (h j) -> p h j", h=H, j=T),
                        axis=mybir.AxisListType.X, op=AOP.add)
                    pst = pp.tile([P, P], f32, tag="pst", name="ppa")
                    nc.tensor.transpose(out=pst[:], in_=hagg[:], identity=ident[:])
                    haggT = sb.tile([P, H], bf16, tag="haggT")
                    nc.vector.tensor_copy(out=haggT[:], in_=pst[:])
                    ps2 = pp2.tile([P, 3 * H], f32, tag="ps2t", name="ps2a")
                    nc.tensor.matmul(ps2[:, 0:H], lhsT=haggT[:], rhs=gatW[k][:], start=True, stop=False)
                    nc.tensor.matmul(ps2[:, 0:H], lhsT=ones_row[:], rhs=gatb[k][:], start=False, stop=True)
                    if k < L - 1:
                        tile_t = sb.tile([P, TC], f32, tag="tab")
                        nc.scalar.activation(tile_t[:, 0:H], ps2[:, 0:H], ACT.Relu)
                        nc.vector.memset(tile_t[:, 130:132], 0.0)
                        score_cols(tile_t[:, 0:H], k + 1, w, tile_t)
                        nc.sync.dma_start(out=own_tab[k + 1][w * P:(w + 1) * P, :],
                                          in_=tile_t[:])
                    else:
                        tile3 = sb.tile([P, TC], f32, tag="t3")
                        nc.scalar.activation(tile3[:, 0:H], ps2[:, 0:H], ACT.Relu)
                        nc.vector.memset(tile3[:, 129:132], 0.0)
                        ps3 = pp.tile([P, P], f32, tag="pst", name="ppa")
                        nc.tensor.transpose(out=ps3[:], in_=tile3[:, 0:H], identity=ident[:])
                        h3T = sb.tile([P, H], bf16, tag="h3T")
                        nc.vector.tensor_copy(out=h3T[:], in_=ps3[:])
                        ps4 = pp2.tile([P, 3 * H], f32, tag="ps2t", name="ps2a")
                        nc.tensor.matmul(ps4[:, 0:H], lhsT=h3T[:], rhs=attW1[:], start=True, stop=False)
                        nc.tensor.matmul(ps4[:, 0:H], lhsT=ones_row[:], rhs=attb1[:], start=False, stop=True)
                        th = sb.tile([P, H], f32, tag="th")
                        nc.scalar.activation(th[:], ps4[:, 0:H], ACT.Tanh)
                        tmp3 = sb.tile([P, H], f32, tag="tmp3")
                        nc.vector.tensor_tensor(out=tmp3[:], in0=th[:], in1=w2rep[:], op=AOP.mult)
                        s_c = sb.tile([P, 1], f32, tag="sc")
                        nc.vector.tensor_reduce(out=s_c[:], in_=tmp3[:], axis=mybir.AxisListType.X, op=AOP.add)
                        nc.scalar.activation(tile3[:, 128:129], s_c[:], ACT.Exp)
                        sc = nc.gpsimd.indirect_dma_start(
                            out=grid3[:, :],
                            out_offset=bass.IndirectOffsetOnAxis(
                                ap=scat_sb[:, w:w + 1], axis=0),
                            in_=tile3[:], in_offset=None)
                        add_dep_helper(sc.ins, scat_ld.ins, reason="scat")
                if k < L - 1:
                    write_sentinel(own_tab[k + 1])
                    nc.gpsimd.collective_compute(
                        "AllGather", AOP.bypass, replica_groups=[list(range(NCORES))],
                        ins=[own_tab[k + 1][:, :].opt()], outs=[tables[k + 1][:, :].opt()])

            if stage == 2:
                dbg2 = sb.tile([P, TC], f32, tag="dbg2")
                nc.sync.dma_start(out=dbg2[:], in_=own_tab[1][0:P, :])
                nc.sync.dma_start(out=out_t[:, :], in_=dbg2[:, 0:1])
            # ---------------- readout ----------------
            if stage >= 4:
                JC = 32
                ctx = sb.tile([P, H], f32, tag="ctx")
                zg = sb.tile([P, 1], f32, tag="zacc")
                grid_v = grid3[0:P * J, :].rearrange("(p j) c -> p j c", p=P, j=J)
                for ci, j0 in enumerate(range(0, J, JC)):
                    jn = min(JC, J - j0)
                    Grc = gp.tile([P, JC, TC], f32, tag="Gr", name="Grc")
                    nc.sync.dma_start(out=Grc[:, 0:jn, :], in_=grid_v[:, j0:j0 + jn, :])
                    zc = sb.tile([P, 1], f32, tag="z")
                    nc.vector.tensor_reduce(out=zc[:], in_=Grc[:, 0:jn, 128],
                                            axis=mybir.AxisListType.X, op=AOP.add)
                    Mg = mp.tile([P, H * JC], f32, tag="Mg", bufs=1, name="Mg")
                    nc.vector.tensor_tensor(
                        out=Mg[:].rearrange("p (h j) -> p j h", j=JC, h=H)[:, 0:jn, :],
                        in0=Grc[:, 0:jn, 0:H],
                        in1=Grc[:, 0:jn, 128].to_broadcast([P, jn, H]),
                        op=AOP.mult)
                    ctxc = sb.tile([P, H], f32, tag="ctxc")
                    nc.vector.tensor_reduce(
                        out=ctxc[:],
                        in_=Mg[:].rearrange("p (h j) -> p h j", h=H, j=JC)[:, :, 0:jn],
                        axis=mybir.AxisListType.X, op=AOP.add)
                    if ci == 0:
                        nc.vector.tensor_copy(out=ctx[:], in_=ctxc[:])
                        nc.vector.tensor_copy(out=zg[:], in_=zc[:])
                    else:
                        nc.vector.tensor_tensor(out=ctx[:], in0=ctx[:], in1=ctxc[:], op=AOP.add)
                        nc.vector.tensor_tensor(out=zg[:], in0=zg[:], in1=zc[:], op=AOP.add)
                nc.vector.tensor_scalar(out=zg[:], in0=zg[:], scalar1=1e-16,
                                        scalar2=None, op0=AOP.add)
                rg = sb.tile([P, 1], f32, tag="r")
                nc.vector.reciprocal(out=rg[:], in_=zg[:])
                nc.vector.tensor_scalar(out=ctx[:], in0=ctx[:], scalar1=rg[:],
                                        scalar2=None, op0=AOP.mult)

                psT = pp.tile([P, P], f32, tag="pst", name="ppa")
                nc.tensor.transpose(out=psT[:], in_=ctx[:], identity=ident[:])
                ctxT = sb.tile([P, H], bf16, tag="ctxT")
                nc.vector.tensor_copy(out=ctxT[:], in_=psT[:])
                gi_ps = pp2.tile([P, 3 * H], f32, tag="ps2t", name="ps2b")
                nc.tensor.matmul(gi_ps[:, :], lhsT=ctxT[:], rhs=WihT[:], start=True, stop=False)
                nc.tensor.matmul(gi_ps[:], lhsT=ones_row[:], rhs=bih[:], start=False, stop=True)
                gi = sb.tile([P, 3 * H], f32, tag="gisb")
                nc.vector.tensor_copy(out=gi[:], in_=gi_ps[:])

                h_st = sb.tile([P, H], f32, tag="hst")
                nc.vector.memset(h_st[:], 0.0)
                for it in range(2):
                    gh = sb.tile([P, 3 * H], f32, tag="ghsb")
                    if it == 0:
                        nc.vector.tensor_copy(out=gh[:], in_=bhhr[:])
                    else:
                        psh = pp.tile([P, P], f32, tag="pst", name="ppa")
                        nc.tensor.transpose(out=psh[:], in_=h_st[:], identity=ident[:])
                        hT = sb.tile([P, H], bf16, tag="hT")
                        nc.vector.tensor_copy(out=hT[:], in_=psh[:])
                        gh_ps = pp2.tile([P, 3 * H], f32, tag="ps2t", name="ps2b")
                        nc.tensor.matmul(gh_ps[:], lhsT=hT[:], rhs=WhhT[:], start=True, stop=False)
                        nc.tensor.matmul(gh_ps[:], lhsT=ones_row[:], rhs=bhh[:], start=False, stop=True)
                        nc.vector.tensor_copy(out=gh[:], in_=gh_ps[:])
                    rr = sb.tile([P, H], f32, tag="rr")
                    nc.vector.tensor_tensor(out=rr[:], in0=gi[:, 0:H], in1=gh[:, 0:H], op=AOP.add)
                    nc.scalar.activation(rr[:], rr[:], ACT.Sigmoid)
                    zz = sb.tile([P, H], f32, tag="zz")
                    nc.vector.tensor_tensor(out=zz[:], in0=gi[:, H:2 * H], in1=gh[:, H:2 * H], op=AOP.add)
                    nc.scalar.activation(zz[:], zz[:], ACT.Sigmoid)
                    nn_ = sb.tile([P, H], f32, tag="nn")
                    nc.vector.tensor_tensor(out=nn_[:], in0=rr[:], in1=gh[:, 2 * H:3 * H], op=AOP.mult)
                    nc.vector.tensor_tensor(out=nn_[:], in0=nn_[:], in1=gi[:, 2 * H:3 * H], op=AOP.add)
                    nc.scalar.activation(nn_[:], nn_[:], ACT.Tanh)
                    omz = sb.tile([P, H], f32, tag="omz")
                    nc.vector.tensor_scalar(out=omz[:], in0=zz[:], scalar1=-1.0,
                                            scalar2=1.0, op0=AOP.mult, op1=AOP.add)
                    nc.vector.tensor_tensor(out=omz[:], in0=omz[:], in1=nn_[:], op=AOP.mult)
                    zh = sb.tile([P, H], f32, tag="zh")
                    nc.vector.tensor_tensor(out=zh[:], in0=zz[:], in1=h_st[:], op=AOP.mult)
                    h_new = sb.tile([P, H], f32, tag="hst")
                    nc.vector.tensor_tensor(out=h_new[:], in0=omz[:], in1=zh[:], op=AOP.add)
                    h_st = h_new

                pso = pp.tile([P, P], f32, tag="pst", name="ppa")
                nc.tensor.transpose(out=pso[:], in_=h_st[:], identity=ident[:])
                hT2 = sb.tile([P, H], bf16, tag="hT2")
                nc.vector.tensor_copy(out=hT2[:], in_=pso[:])
                o1_ps = pp2.tile([P, 3 * H], f32, tag="ps2t", name="ps2a")
                nc.tensor.matmul(o1_ps[:, 0:H], lhsT=hT2[:], rhs=W1o[:], start=True, stop=False)
                nc.tensor.matmul(o1_ps[:, 0:H], lhsT=ones_row[:], rhs=b1o[:], start=False, stop=True)
                o1 = sb.tile([P, H], f32, tag="o1")
                nc.scalar.activation(o1[:], o1_ps[:, 0:H], ACT.Relu)
                pso1 = pp.tile([P, P], f32, tag="pst", name="ppa")
                nc.tensor.transpose(out=pso1[:], in_=o1[:], identity=ident[:])
                o1T = sb.tile([P, H], bf16, tag="o1T")
                nc.vector.tensor_copy(out=o1T[:], in_=pso1[:])
                o2_ps = pp2.tile([P, 3 * H], f32, tag="ps2t", name="ps2c")
                nc.tensor.matmul(o2_ps[:, 0:64], lhsT=o1T[:], rhs=W2o[:], start=True, stop=False)
                nc.tensor.matmul(o2_ps[:, 0:64], lhsT=ones_row[:], rhs=b2o[:], start=False, stop=True)
                o2 = sb.tile([P, 64], f32, tag="o2")
                nc.scalar.activation(o2[:], o2_ps[:, 0:64], ACT.Relu)
                pso2 = pp.tile([P, P], f32, tag="pst", name="ppb")
                nc.tensor.transpose(out=pso2[0:64, 0:P], in_=o2[:], identity=ident[:])
                o2T = sb.tile([64, P], bf16, tag="o2T")
                nc.vector.tensor_copy(out=o2T[:], in_=pso2[0:64, 0:P])
                o3_ps = pp2.tile([P, 3 * H], f32, tag="ps2t", name="ps2d")
                nc.tensor.matmul(o3_ps[:, 0:1], lhsT=o2T[:], rhs=W3o[:], start=True, stop=False)
                nc.tensor.matmul(o3_ps[:, 0:1], lhsT=ones_row[:], rhs=b3o[:], start=False, stop=True)
                osig = sb.tile([P, 1], f32, tag="osig")
                nc.scalar.activation(osig[:], o3_ps[:, 0:1], ACT.Sigmoid)
                nc.sync.dma_start(out=out_t[:, :], in_=osig[:])
    nc.compile()
    # the BIR is frozen after compile; memoize its (deterministic) JSON
    # serialization so repeated run_bass_kernel_spmd calls skip re-encoding
    # ~5k instructions during jit lowering
    bir_json = nc.to_json_bytes()
    nc.to_json_bytes = lambda: bir_json
    return nc


def _run(nc, in_maps, trace=False):
    from concourse.bass_utils import run_bass_kernel_spmd

    res = run_bass_kernel_spmd(nc, in_maps, core_ids=list(range(NCORES)), trace=trace)
    return np.concatenate([res.results[c]["out"][:, 0] for c in range(NCORES)]), res


def kernel(x, edge_index, batch, enc_W, enc_b, gat_W, gat_a_src, gat_a_dst, gat_b,
           att_W1, att_b1, att_w2, att_b2, gru_Wih, gru_Whh, gru_bih, gru_bhh,
           out_W1, out_b1, out_W2, out_b2, out_W3, out_b3):
    x = np.asarray(x, np.float32)
    edge_index = np.asarray(edge_index)
    batch = np.asarray(batch).astype(np.int64)
    meta, offs, scat, xT = _build_host(x, edge_index, batch)
    nc = _build_bass(meta)
    weights = _pack_weights(enc_W, enc_b, gat_W, gat_a_src, gat_a_dst, gat_b,
                            att_W1, att_b1, att_w2, att_b2, gru_Wih, gru_Whh,
                            gru_bih, gru_bhh, out_W1, out_b1, out_W2, out_b2,
                            out_W3, out_b3)
    in_maps = _make_inmaps(meta, offs, scat, xT, weights)
    out, _ = _run(nc, in_maps)
    return out.astype(np.float32)


# revision 34
# speedup vs baseline: 1.0897x; 1.0897x over previous
"""AttentiveFP (3x GAT + segment-softmax GRU readout) on 8 Trainium2 cores.

Sharding: data-parallel over graphs. Core c owns graphs [128c, 128(c+1)) and
their nodes (batch is sorted). Per GAT layer each core aggregates messages for
its own destination nodes (dst-major degree-bucketed grid), then the per-node
feature table (h | exp(a_src) | exp(0.2 a_src), fp32) is AllGathered so every
core can gather arbitrary source rows next layer. The GAT aggregation is done
on h directly (sum_e alpha_e h[src] then @W afterwards - linearity), and
exp(leaky(a_s+a_d)) is computed separably as max(e^as * e^ad, e^.2as * e^.2ad)
so edge scores need no per-edge cross terms. Readout: graph-major grid,
softmax + context are per-partition ops; GRU runs twice on a loop-invariant
context; output MLP on [128 graphs, H] per core.

Host->device traffic is minimized (the runs are PJRT-proxied over a slow
tunnel, so input bytes + per-call jit overhead dominate wall time, not device
compute): node features ship as fp8-e3m4 (~3.3MB), weights as one fp8 blob
scaled x16 into e3m4's normal range (un-scaled during the on-device upconvert
to bf16), gather/scatter indices as one uint16 blob (cast to int32 by a SWDGE
DMA on device), and only the exp-amplified attention row vectors stay f32.
Replicated per-partition weight tiles are built on device with a ones-column
matmul instead of being shipped 128x. The XLA executable is cached across
calls via the jax persistent compilation cache, and the BIR JSON serialization
is memoized on the Bass instance (it is re-encoded inside every jit lowering
otherwise). Per-window gathers stay one-indirect-DMA-per-degree-slot: the HW
ucode only honors a single offset column per partition (batched multi-column
offset APs silently misaddress), and dma_gather needs int16 row ids which a
52k-row table cannot satisfy.
"""

import numpy as np
import ml_dtypes

try:  # cache the XLA executable across run_bass_kernel_spmd calls
    import jax as _jax

    _jax.config.update("jax_compilation_cache_dir", "/tmp/jax_comp_cache")
    _jax.config.update("jax_persistent_cache_min_entry_size_bytes", -1)
    _jax.config.update("jax_persistent_cache_min_compile_time_secs", 0.0)
except Exception:
    pass

BF16 = np.dtype(ml_dtypes.bfloat16)
F8 = np.dtype(ml_dtypes.float8_e3m4)

H = 128
L = 3
P = 128
NCORES = 8
TC = 132  # table row: h(128) | exp(a_s) | exp(.2 a_s) | pad | pad


def _wb_layout(meta):
    """fp8 (e3m4) blob layout: name -> (elem offset, rows, cols)."""
    segs = [("encW", 64, H)]
    for k in range(L):
        segs.append((f"gatW{k}", H, H))
    segs += [("attW1", H, H), ("WihT", H, 3 * H), ("WhhT", H, 3 * H),
             ("W1o", H, H), ("W2o", H, 64), ("W3o", 64, 1), ("encb", 1, H)]
    for k in range(L):
        segs.append((f"gatb{k}", 1, H))
    segs += [("attb1", 1, H), ("w2", 1, H), ("bih", 1, 3 * H), ("bhh", 1, 3 * H),
             ("b1o", 1, H), ("b2o", 1, 64), ("b3o", 1, 1)]
    out, cur = {}, 0
    for name, r, c in segs:
        out[name] = (cur, r, c)
        cur += r * c
    cur = ((cur + NCORES - 1) // NCORES) * NCORES  # pad so the blob shards 8-way
    return out, cur


def _fb_layout():
    """f32 blob: attention score rows (exp-amplified, keep full precision)."""
    out, cur = {}, 0
    for k in range(L):
        out[f"ws{k}"] = (cur, 1, H)
        cur += H
    for k in range(L):
        out[f"wd{k}"] = (cur, 1, H)
        cur += H
    return out, cur


def _build_host(x, edge_index, batch):
    N = x.shape[0]
    G = 1024
    gpc = G // NCORES

    src = np.concatenate([np.arange(N, dtype=np.int64), edge_index[0].astype(np.int64)])
    dst = np.concatenate([np.arange(N, dtype=np.int64), edge_index[1].astype(np.int64)])

    node_core = (batch // gpc).astype(np.int64)
    S = np.bincount(node_core, minlength=NCORES)
    indeg = np.bincount(dst, minlength=N)

    # order nodes within each core by in-degree descending
    node_pos = np.zeros(N, np.int64)
    perm = []
    for c in range(NCORES):
        cn = np.where(node_core == c)[0]
        order = cn[np.argsort(-indeg[cn], kind="stable")]
        perm.append(order)
        node_pos[order] = np.arange(len(order))

    NW = int(max((s + P - 1) // P for s in S))
    S_pad = (NW + 1) * P
    R_tot = NCORES * S_pad
    sent_row = np.array([c * S_pad + NW * P for c in range(NCORES)], np.int64)
    table_row = node_core * S_pad + node_pos

    # per-window max in-window degree, equalized across cores
    win_of_node = node_pos // P
    Tw = np.zeros(NW, np.int64)
    np.maximum.at(Tw, win_of_node, indeg)
    Tw = np.maximum(Tw, 1)
    SUMT = int(Tw.sum())
    t_off = np.concatenate([[0], np.cumsum(Tw)]).astype(np.int64)

    # j-rank of each edge within its dst (vectorized)
    order_e = np.argsort(dst, kind="stable")
    ds = dst[order_e]
    starts = np.concatenate([[0], np.cumsum(np.bincount(ds, minlength=N))[:-1]])
    jrank = np.arange(len(ds)) - starts[ds]

    e_core = node_core[ds]
    dloc = node_pos[ds]
    col = t_off[dloc // P] + jrank
    row = dloc % P
    assert R_tot < 65536
    offs = np.empty((NCORES, P, SUMT), np.uint16)
    for c in range(NCORES):
        offs[c, :, :] = sent_row[c]
        sel = e_core == c
        offs[c, row[sel], col[sel]] = table_row[src[order_e][sel]]

    # readout grid
    gsizes = np.bincount(batch, minlength=G)
    J = int(gsizes.max())
    gstarts = np.concatenate([[0], np.cumsum(gsizes)[:-1]])
    j_in_graph = np.arange(N) - gstarts[batch]
    g_loc = (batch - node_core * gpc).astype(np.int64)
    grid_rows = gpc * J + 1
    junk_row = gpc * J
    assert grid_rows < 65536
    scat = np.full((NCORES, P, NW), junk_row, np.uint16)
    for c in range(NCORES):
        idx = np.arange(len(perm[c]))
        scat[c, idx % P, idx // P] = g_loc[perm[c]] * J + j_in_graph[perm[c]]
    # column t_off[w]+0 of each window is gather-dead (slot j=0 is the dst's
    # own row, served from own_tab, and jrank 0 is always the self-loop) --
    # carry the readout scatter indices there instead of a separate region
    for w in range(NW):
        offs[:, :, t_off[w]] = scat[:, :, w]

    xT = np.zeros((NCORES, x.shape[1], NW * P), F8)
    for c in range(NCORES):
        xT[c, :, :S[c]] = x[perm[c]].T.astype(F8)

    return dict(NW=NW, S_pad=S_pad, R_tot=R_tot, Tw=Tw, SUMT=SUMT,
                t_off=t_off, J=J, GR=grid_rows, gpc=gpc), offs, scat, xT


def _pack_weights(enc_W, enc_b, gat_W, gat_a_src, gat_a_dst, gat_b,
                  att_W1, att_b1, att_w2, att_b2, gru_Wih, gru_Whh,
                  gru_bih, gru_bhh, out_W1, out_b1, out_W2, out_b2,
                  out_W3, out_b3):
    f = lambda a: np.asarray(a, np.float32)
    w = dict(
        encW=f(enc_W), encb=f(enc_b),
        attW1=f(att_W1), attb1=f(att_b1), w2=f(att_w2),
        WihT=np.ascontiguousarray(f(gru_Wih).T),
        WhhT=np.ascontiguousarray(f(gru_Whh).T),
        bih=f(gru_bih), bhh=f(gru_bhh),
        W1o=f(out_W1), b1o=f(out_b1),
        W2o=f(out_W2), b2o=f(out_b2),
        W3o=f(out_W3), b3o=f(out_b3).reshape(1),
    )
    for k in range(L):
        w[f"gatW{k}"] = f(gat_W[k])
        w[f"gatb{k}"] = f(gat_b[k])
        w[f"ws{k}"] = f(gat_W[k]) @ f(gat_a_src[k])
        w[f"wd{k}"] = f(gat_W[k]) @ f(gat_a_dst[k])
    return w


def _make_inmaps(meta, offs, scat, xT, weights):
    woff, nbw = _wb_layout(meta)
    foff, nbf = _fb_layout()
    SUMT, NW = meta["SUMT"], meta["NW"]

    # weights are ~0.08-scale: x16 keeps them in e3m4's normal range (the
    # device multiplies by 1/16 during the fp8->bf16 upconvert)
    wb = np.zeros(nbw, F8)
    for name, (a, r, c) in woff.items():
        wb[a:a + r * c] = (weights[name] * 16.0).astype(F8).reshape(-1)
    fb = np.zeros(nbf, np.float32)
    for name, (a, r, c) in foff.items():
        fb[a:a + r * c] = weights[name].reshape(-1)

    # each core ships 1/8th of the (identical) weight blob; the kernel
    # AllGathers the full blob on device before unpacking
    s8 = nbw // NCORES
    in_maps = []
    for c in range(NCORES):
        in_maps.append(dict(wb=wb[c * s8:(c + 1) * s8], ib=offs[c].reshape(-1),
                            fb=fb, xb=xT[c].reshape(-1)))
    return in_maps


def _build_bass(meta, stage=99):
    import concourse.bass as bass
    import concourse.mybir as mybir
    import concourse.tile as tile
    import concourse.bacc as bacc
    from concourse.tile import add_dep_helper
    from concourse.masks import make_identity

    f32 = mybir.dt.float32
    bf16 = mybir.dt.bfloat16
    f8 = mybir.dt.float8e3
    u16 = mybir.dt.uint16
    i32 = mybir.dt.int32
    AOP = mybir.AluOpType
    ACT = mybir.ActivationFunctionType
    NW, S_pad, R_tot = meta["NW"], meta["S_pad"], meta["R_tot"]
    Tw, t_off, J, GR = meta["Tw"], meta["t_off"], meta["J"], meta["GR"]
    SUMT = meta["SUMT"]

    woff, nbw = _wb_layout(meta)
    foff, nbf = _fb_layout()

    nc = bacc.Bacc("TRN2", target_bir_lowering=False, debug=False, num_devices=NCORES)

    s8 = nbw // NCORES
    wb_in = nc.dram_tensor("wb", [s8], f8, kind="ExternalInput")
    ib_in = nc.dram_tensor("ib", [P * SUMT], u16, kind="ExternalInput")
    fb_in = nc.dram_tensor("fb", [nbf], f32, kind="ExternalInput")
    xb_in = nc.dram_tensor("xb", [64 * NW * P], f8, kind="ExternalInput")
    wb_stage = nc.dram_tensor("wb_stage", [s8], f8)
    wb_full = nc.dram_tensor("wb_full", [nbw], f8)
    out_t = nc.dram_tensor("out", [P, 1], f32, kind="ExternalOutput")

    own_tab = [nc.dram_tensor(f"own{k}", [S_pad, TC], f32) for k in range(L)]
    tables = [nc.dram_tensor(f"tab{k}", [R_tot, TC], f32) for k in range(L)]
    grid3 = nc.dram_tensor("grid3", [GR, TC], f32)

    def wview(name):
        a, r, c = woff[name]
        return wb_full[a:a + r * c].rearrange("(r c) -> r c", r=r, c=c)

    def fview(name):
        a, r, c = foff[name]
        return fb_in[a:a + r * c].rearrange("(r c) -> r c", r=r, c=c)

    with tile.TileContext(nc) as tc:
        with (
            tc.tile_pool(name="const", bufs=1) as cp,
            tc.tile_pool(name="sb", bufs=3) as sb,
            tc.tile_pool(name="gth", bufs=2) as gp,
            tc.tile_pool(name="mbuf", bufs=2) as mp,
            tc.tile_pool(name="ps", bufs=2, space="PSUM") as pp,
            tc.tile_pool(name="ps2", bufs=2, space="PSUM") as pp2,
        ):
            nc.sync.dma_start(out=wb_stage[:], in_=wb_in[:])
            nc.gpsimd.collective_compute(
                "AllGather", AOP.bypass, replica_groups=[list(range(NCORES))],
                ins=[wb_stage[:].opt()], outs=[wb_full[:].opt()])

            ident = cp.tile([P, P], f32)
            make_identity(nc, ident[:])
            ones_row = cp.tile([1, P], bf16)
            nc.vector.memset(ones_row[:], 1.0)
            ones_f32 = cp.tile([1, P], f32)
            nc.vector.memset(ones_f32[:], 1.0)

            _dc = [0]

            def wtile(name):
                a, r, c = woff[name]
                _dc[0] += 1
                t8 = cp.tile([r, c], f8, tag=f"c{_dc[0]}q", name=f"c{_dc[0]}q")
                nc.sync.dma_start(out=t8[:], in_=wview(name))
                t = cp.tile([r, c], bf16, tag=f"c{_dc[0]}", name=f"c{_dc[0]}")
                nc.vector.tensor_scalar(out=t[:], in0=t8[:], scalar1=0.0625,
                                        scalar2=None, op0=AOP.mult)
                return t

            def reprow(rowtile, n, dt):
                """[1,n] SBUF row tile -> [P,n] replicated f32 SBUF tile."""
                _dc[0] += 1
                ps = pp2.tile([P, 3 * H], f32, tag="ps2t", name="ps2r")
                nc.tensor.matmul(ps[:, 0:n],
                                 lhsT=(ones_f32 if dt == f32 else ones_row)[:],
                                 rhs=rowtile[:], start=True, stop=True)
                t = cp.tile([P, n], f32, tag=f"c{_dc[0]}r", name=f"c{_dc[0]}r")
                nc.vector.tensor_copy(out=t[:], in_=ps[:, 0:n])
                return t

            def frow(name):
                a, r, c = foff[name]
                _dc[0] += 1
                t = cp.tile([1, c], f32, tag=f"c{_dc[0]}", name=f"c{_dc[0]}")
                nc.sync.dma_start(out=t[:], in_=fview(name))
                return t

            offs_sb = cp.tile([P, SUMT], i32)
            offs_ld = nc.gpsimd.dma_start(
                out=offs_sb[:],
                in_=ib_in[0:P * SUMT].rearrange("(r c) -> r c", r=P, c=SUMT))

            x8 = cp.tile([64, NW * P], f8)
            nc.sync.dma_start(out=x8[:], in_=xb_in[:].rearrange("(r c) -> r c", r=64, c=NW * P))
            xall = cp.tile([64, NW * P], bf16)
            nc.vector.tensor_copy(out=xall[:], in_=x8[:])
            encW = wtile("encW")
            encb = wtile("encb")
            gatW = [wtile(f"gatW{k}") for k in range(L)]
            gatb = [wtile(f"gatb{k}") for k in range(L)]
            attW1 = wtile("attW1")
            attb1 = wtile("attb1")
            WihT = wtile("WihT")
            WhhT = wtile("WhhT")
            bih = wtile("bih")
            bhh = wtile("bhh")
            W1o = wtile("W1o")
            b1o = wtile("b1o")
            W2o = wtile("W2o")
            b2o = wtile("b2o")
            W3o = wtile("W3o")
            b3o = wtile("b3o")

            wsrep = [reprow(frow(f"ws{k}"), H, f32) for k in range(L)]
            wdrep = [reprow(frow(f"wd{k}"), H, f32) for k in range(L)]
            w2rep = reprow(wtile("w2"), H, bf16)
            bhhr = reprow(wtile("bhh"), 3 * H, bf16)

            Bbuf = [cp.tile([P, NW], f32, tag=f"B{k}", name=f"Bbuf{k}") for k in range(L)]
            B2buf = [cp.tile([P, NW], f32, tag=f"B2{k}", name=f"B2buf{k}") for k in range(L)]

            def score_cols(h_view, k, w, tile_t):
                tmp = sb.tile([P, H], f32, tag="ttrtmp")
                nc.vector.tensor_tensor(out=tmp[:], in0=h_view, in1=wsrep[k][:], op=AOP.mult)
                a_s = sb.tile([P, 1], f32, tag="as")
                nc.vector.tensor_reduce(out=a_s[:], in_=tmp[:], axis=mybir.AxisListType.X, op=AOP.add)
                nc.scalar.activation(tile_t[:, 128:129], a_s[:], ACT.Exp)
                nc.scalar.activation(tile_t[:, 129:130], a_s[:], ACT.Exp, scale=0.2)
                tmp2 = sb.tile([P, H], f32, tag="ttrtmp2")
                nc.vector.tensor_tensor(out=tmp2[:], in0=h_view, in1=wdrep[k][:], op=AOP.mult)
                a_d = sb.tile([P, 1], f32, tag="ad")
                nc.vector.tensor_reduce(out=a_d[:], in_=tmp2[:], axis=mybir.AxisListType.X, op=AOP.add)
                nc.scalar.activation(Bbuf[k][:, w:w + 1], a_d[:], ACT.Exp)
                nc.scalar.activation(B2buf[k][:, w:w + 1], a_d[:], ACT.Exp, scale=0.2)

            def write_sentinel(tab):
                z = sb.tile([P, TC], f32, tag="tab")
                nc.vector.memset(z[:], 0.0)
                nc.sync.dma_start(out=tab[NW * P:(NW + 1) * P, :], in_=z[:])

            # ---------------- encoder ----------------
            for w in range(NW):
                ps = pp.tile([P, P], f32, tag="pst", name="ppa")
                nc.tensor.matmul(ps[:], lhsT=xall[:, w * P:(w + 1) * P], rhs=encW[:],
                                 start=True, stop=False)
                nc.tensor.matmul(ps[:], lhsT=ones_row[:], rhs=encb[:], start=False, stop=True)
                tile_t = sb.tile([P, TC], f32, tag="tab")
                nc.scalar.activation(tile_t[:, 0:H], ps[:], ACT.Relu)
                nc.vector.memset(tile_t[:, 130:132], 0.0)
                score_cols(tile_t[:, 0:H], 0, w, tile_t)
                nc.sync.dma_start(out=own_tab[0][w * P:(w + 1) * P, :], in_=tile_t[:])
            write_sentinel(own_tab[0])

            if stage >= 1:
                nc.gpsimd.collective_compute(
                    "AllGather", AOP.bypass, replica_groups=[list(range(NCORES))],
                    ins=[own_tab[0][:, :].opt()], outs=[tables[0][:, :].opt()])

            if stage <= 1:
                dbg = sb.tile([P, 1], f32, tag="dbg")
                nc.sync.dma_start(out=dbg[:], in_=(tables[0] if stage >= 1 else own_tab[0])[0:P, 0:1])
                nc.sync.dma_start(out=out_t[:, :], in_=dbg[:])
            zg_t = cp.tile([P, TC], f32)
            nc.vector.memset(zg_t[:], 0.0)
            for b in range((GR + P - 1) // P):
                lo = b * P
                hi = min(GR, lo + P)
                nc.sync.dma_start(out=grid3[lo:hi, :], in_=zg_t[0:hi - lo, :])

            # ---------------- GAT layers ----------------
            for k in range(L if stage >= 3 else (1 if stage == 2 else 0)):
                for w in range(NW):
                    T = int(Tw[w])
                    Gw = gp.tile([P, T, TC], f32, tag="G", bufs=3)
                    nc.sync.dma_start(out=Gw[:, 0, :],
                                      in_=own_tab[k][w * P:(w + 1) * P, :])
                    for j in range(1, T):
                        gi = nc.gpsimd.indirect_dma_start(
                            out=Gw[:, j, :], out_offset=None,
                            in_=tables[k][:, :],
                            in_offset=bass.IndirectOffsetOnAxis(
                                ap=offs_sb[:, t_off[w] + j:t_off[w] + j + 1],
                                axis=0))
                        add_dep_helper(gi.ins, offs_ld.ins, reason="offs")
                    u1 = sb.tile([P, T], f32, tag="u1")
                    nc.vector.tensor_scalar(out=u1[:], in0=Gw[:, :, 128],
                                            scalar1=Bbuf[k][:, w:w + 1],
                                            scalar2=None, op0=AOP.mult)
                    u2 = sb.tile([P, T], f32, tag="u2")
                    nc.vector.tensor_scalar(out=u2[:], in0=Gw[:, :, 129],
                                            scalar1=B2buf[k][:, w:w + 1],
                                            scalar2=None, op0=AOP.mult)
                    wt = sb.tile([P, T], f32, tag="wt")
                    nc.vector.tensor_tensor(out=wt[:], in0=u1[:], in1=u2[:], op=AOP.max)
                    z = sb.tile([P, 1], f32, tag="z")
                    nc.vector.tensor_reduce(out=z[:], in_=wt[:],
                                            axis=mybir.AxisListType.X, op=AOP.add)
                    nc.vector.tensor_scalar(out=z[:], in0=z[:], scalar1=1e-16,
                                            scalar2=None, op0=AOP.add)
                    r = sb.tile([P, 1], f32, tag="r")
                    nc.vector.reciprocal(out=r[:], in_=z[:])
                    al = sb.tile([P, T], f32, tag="al")
                    nc.vector.tensor_scalar(out=al[:], in0=wt[:], scalar1=r[:],
                                            scalar2=None, op0=AOP.mult)
                    M = mp.tile([P, H * T], f32, tag="M", bufs=1)
                    nc.vector.tensor_tensor(
                        out=M[:].rearrange("p (h j) -> p j h", j=T, h=H),
                        in0=Gw[:, :, 0:H],
                        in1=al[:].to_broadcast([P, T, H]),
                        op=AOP.mult)
                    hagg = sb.tile([P, H], f32, tag="hagg")
                    nc.vector.tensor_reduce(
                        out=hagg[:], in_=M[:].rearrange("p (h j) -> p h j", h=H, j=T),
                        axis=mybir.AxisListType.X, op=AOP.add)
                    pst = pp.tile([P, P], f32, tag="pst", name="ppa")
                    nc.tensor.transpose(out=pst[:], in_=hagg[:], identity=ident[:])
                    haggT = sb.tile([P, H], bf16, tag="haggT")
                    nc.vector.tensor_copy(out=haggT[:], in_=pst[:])
                    ps2 = pp2.tile([P, 3 * H], f32, tag="ps2t", name="ps2a")
                    nc.tensor.matmul(ps2[:, 0:H], lhsT=haggT[:], rhs=gatW[k][:], start=True, stop=False)
                    nc.tensor.matmul(ps2[:, 0:H], lhsT=ones_row[:], rhs=gatb[k][:], start=False, stop=True)
                    if k < L - 1:
                        tile_t = sb.tile([P, TC], f32, tag="tab")
                        nc.scalar.activation(tile_t[:, 0:H], ps2[:, 0:H], ACT.Relu)
                        nc.vector.memset(tile_t[:, 130:132], 0.0)
                        score_cols(tile_t[:, 0:H], k + 1, w, tile_t)
                        nc.sync.dma_start(out=own_tab[k + 1][w * P:(w + 1) * P, :],
                                          in_=tile_t[:])
                    else:
                        tile3 = sb.tile([P, TC], f32, tag="t3")
                        nc.scalar.activation(tile3[:, 0:H], ps2[:, 0:H], ACT.Relu)
                        nc.vector.memset(tile3[:, 129:132], 0.0)
                        ps3 = pp.tile([P, P], f32, tag="pst", name="ppa")
                        nc.tensor.transpose(out=ps3[:], in_=tile3[:, 0:H], identity=ident[:])
                        h3T = sb.tile([P, H], bf16, tag="h3T")
                        nc.vector.tensor_copy(out=h3T[:], in_=ps3[:])
                        ps4 = pp2.tile([P, 3 * H], f32, tag="ps2t", name="ps2a")
                        nc.tensor.matmul(ps4[:, 0:H], lhsT=h3T[:], rhs=attW1[:], start=True, stop=False)
                        nc.tensor.matmul(ps4[:, 0:H], lhsT=ones_row[:], rhs=attb1[:], start=False, stop=True)
                        th = sb.tile([P, H], f32, tag="th")
                        nc.scalar.activation(th[:], ps4[:, 0:H], ACT.Tanh)
                        tmp3 = sb.tile([P, H], f32, tag="tmp3")
                        nc.vector.tensor_tensor(out=tmp3[:], in0=th[:], in1=w2rep[:], op=AOP.mult)
                        s_c = sb.tile([P, 1], f32, tag="sc")
                        nc.vector.tensor_reduce(out=s_c[:], in_=tmp3[:], axis=mybir.AxisListType.X, op=AOP.add)
                        nc.scalar.activation(tile3[:, 128:129], s_c[:], ACT.Exp)
                        sc = nc.gpsimd.indirect_dma_start(
                            out=grid3[:, :],
                            out_offset=bass.IndirectOffsetOnAxis(
                                ap=offs_sb[:, t_off[w]:t_off[w] + 1], axis=0),
                            in_=tile3[:], in_offset=None)
                        add_dep_helper(sc.ins, offs_ld.ins, reason="scat")
                if k < L - 1:
                    write_sentinel(own_tab[k + 1])
                    nc.gpsimd.collective_compute(
                        "AllGather", AOP.bypass, replica_groups=[list(range(NCORES))],
                        ins=[own_tab[k + 1][:, :].opt()], outs=[tables[k + 1][:, :].opt()])

            if stage == 2:
                dbg2 = sb.tile([P, TC], f32, tag="dbg2")
                nc.sync.dma_start(out=dbg2[:], in_=own_tab[1][0:P, :])
                nc.sync.dma_start(out=out_t[:, :], in_=dbg2[:, 0:1])
            # ---------------- readout ----------------
            if stage >= 4:
                JC = 32
                ctx = sb.tile([P, H], f32, tag="ctx")
                zg = sb.tile([P, 1], f32, tag="zacc")
                grid_v = grid3[0:P * J, :].rearrange("(p j) c -> p j c", p=P, j=J)
                for ci, j0 in enumerate(range(0, J, JC)):
                    jn = min(JC, J - j0)
                    Grc = gp.tile([P, JC, TC], f32, tag="Gr", name="Grc")
                    nc.sync.dma_start(out=Grc[:, 0:jn, :], in_=grid_v[:, j0:j0 + jn, :])
                    zc = sb.tile([P, 1], f32, tag="z")
                    nc.vector.tensor_reduce(out=zc[:], in_=Grc[:, 0:jn, 128],
                                            axis=mybir.AxisListType.X, op=AOP.add)
                    Mg = mp.tile([P, H * JC], f32, tag="Mg", bufs=1, name="Mg")
                    nc.vector.tensor_tensor(
                        out=Mg[:].rearrange("p (h j) -> p j h", j=JC, h=H)[:, 0:jn, :],
                        in0=Grc[:, 0:jn, 0:H],
                        in1=Grc[:, 0:jn, 128].to_broadcast([P, jn, H]),
                        op=AOP.mult)
                    ctxc = sb.tile([P, H], f32, tag="ctxc")
                    nc.vector.tensor_reduce(
                        out=ctxc[:],
                        in_=Mg[:].rearrange("p (h j) -> p h j", h=H, j=JC)[:, :, 0:jn],
                        axis=mybir.AxisListType.X, op=AOP.add)
                    if ci == 0:
                        nc.vector.tensor_copy(out=ctx[:], in_=ctxc[:])
                        nc.vector.tensor_copy(out=zg[:], in_=zc[:])
                    else:
                        nc.vector.tensor_tensor(out=ctx[:], in0=ctx[:], in1=ctxc[:], op=AOP.add)
                        nc.vector.tensor_tensor(out=zg[:], in0=zg[:], in1=zc[:], op=AOP.add)
                nc.vector.tensor_scalar(out=zg[:], in0=zg[:], scalar1=1e-16,
                                        scalar2=None, op0=AOP.add)
                rg = sb.tile([P, 1], f32, tag="r")
                nc.vector.reciprocal(out=rg[:], in_=zg[:])
                nc.vector.tensor_scalar(out=ctx[:], in0=ctx[:], scalar1=rg[:],
                                        scalar2=None, op0=AOP.mult)

                psT = pp.tile([P, P], f32, tag="pst", name="ppa")
                nc.tensor.transpose(out=psT[:], in_=ctx[:], identity=ident[:])
                ctxT = sb.tile([P, H], bf16, tag="ctxT")
                nc.vector.tensor_copy(out=ctxT[:], in_=psT[:])
                gi_ps = pp2.tile([P, 3 * H], f32, tag="ps2t", name="ps2b")
                nc.tensor.matmul(gi_ps[:, :], lhsT=ctxT[:], rhs=WihT[:], start=True, stop=False)
                nc.tensor.matmul(gi_ps[:], lhsT=ones_row[:], rhs=bih[:], start=False, stop=True)
                gi = sb.tile([P, 3 * H], f32, tag="gisb")
                nc.vector.tensor_copy(out=gi[:], in_=gi_ps[:])

                h_st = sb.tile([P, H], f32, tag="hst")
                nc.vector.memset(h_st[:], 0.0)
                for it in range(2):
                    gh = sb.tile([P, 3 * H], f32, tag="ghsb")
                    if it == 0:
                        nc.vector.tensor_copy(out=gh[:], in_=bhhr[:])
                    else:
                        psh = pp.tile([P, P], f32, tag="pst", name="ppa")
                        nc.tensor.transpose(out=psh[:], in_=h_st[:], identity=ident[:])
                        hT = sb.tile([P, H], bf16, tag="hT")
                        nc.vector.tensor_copy(out=hT[:], in_=psh[:])
                        gh_ps = pp2.tile([P, 3 * H], f32, tag="ps2t", name="ps2b")
                        nc.tensor.matmul(gh_ps[:], lhsT=hT[:], rhs=WhhT[:], start=True, stop=False)
                        nc.tensor.matmul(gh_ps[:], lhsT=ones_row[:], rhs=bhh[:], start=False, stop=True)
                        nc.vector.tensor_copy(out=gh[:], in_=gh_ps[:])
                    rr = sb.tile([P, H], f32, tag="rr")
                    nc.vector.tensor_tensor(out=rr[:], in0=gi[:, 0:H], in1=gh[:, 0:H], op=AOP.add)
                    nc.scalar.activation(rr[:], rr[:], ACT.Sigmoid)
                    zz = sb.tile([P, H], f32, tag="zz")
                    nc.vector.tensor_tensor(out=zz[:], in0=gi[:, H:2 * H], in1=gh[:, H:2 * H], op=AOP.add)
                    nc.scalar.activation(zz[:], zz[:], ACT.Sigmoid)
                    nn_ = sb.tile([P, H], f32, tag="nn")
                    nc.vector.tensor_tensor(out=nn_[:], in0=rr[:], in1=gh[:, 2 * H:3 * H], op=AOP.mult)
                    nc.vector.tensor_tensor(out=nn_[:], in0=nn_[:], in1=gi[:, 2 * H:3 * H], op=AOP.add)
                    nc.scalar.activation(nn_[:], nn_[:], ACT.Tanh)
                    omz = sb.tile([P, H], f32, tag="omz")
                    nc.vector.tensor_scalar(out=omz[:], in0=zz[:], scalar1=-1.0,
                                            scalar2=1.0, op0=AOP.mult, op1=AOP.add)
                    nc.vector.tensor_tensor(out=omz[:], in0=omz[:], in1=nn_[:], op=AOP.mult)
                    zh = sb.tile([P, H], f32, tag="zh")
                    nc.vector.tensor_tensor(out=zh[:], in0=zz[:], in1=h_st[:], op=AOP.mult)
                    h_new = sb.tile([P, H], f32, tag="hst")
                    nc.vector.tensor_tensor(out=h_new[:], in0=omz[:], in1=zh[:], op=AOP.add)
                    h_st = h_new

                pso = pp.tile([P, P], f32, tag="pst", name="ppa")
                nc.tensor.transpose(out=pso[:], in_=h_st[:], identity=ident[:])
                hT2 = sb.tile([P, H], bf16, tag="hT2")
                nc.vector.tensor_copy(out=hT2[:], in_=pso[:])
                o1_ps = pp2.tile([P, 3 * H], f32, tag="ps2t", name="ps2a")
                nc.tensor.matmul(o1_ps[:, 0:H], lhsT=hT2[:], rhs=W1o[:], start=True, stop=False)
                nc.tensor.matmul(o1_ps[:, 0:H], lhsT=ones_row[:], rhs=b1o[:], start=False, stop=True)
                o1 = sb.tile([P, H], f32, tag="o1")
                nc.scalar.activation(o1[:], o1_ps[:, 0:H], ACT.Relu)
                pso1 = pp.tile([P, P], f32, tag="pst", name="ppa")
                nc.tensor.transpose(out=pso1[:], in_=o1[:], identity=ident[:])
                o1T = sb.tile([P, H], bf16, tag="o1T")
                nc.vector.tensor_copy(out=o1T[:], in_=pso1[:])
                o2_ps = pp2.tile([P, 3 * H], f32, tag="ps2t", name="ps2c")
                nc.tensor.matmul(o2_ps[:, 0:64], lhsT=o1T[:], rhs=W2o[:], start=True, stop=False)
                nc.tensor.matmul(o2_ps[:, 0:64], lhsT=ones_row[:], rhs=b2o[:], start=False, stop=True)
                o2 = sb.tile([P, 64], f32, tag="o2")
                nc.scalar.activation(o2[:], o2_ps[:, 0:64], ACT.Relu)
                pso2 = pp.tile([P, P], f32, tag="pst", name="ppb")
                nc.tensor.transpose(out=pso2[0:64, 0:P], in_=o2[:], identity=ident[:])
                o2T = sb.tile([64, P], bf16, tag="o2T")
                nc.vector.tensor_copy(out=o2T[:], in_=pso2[0:64, 0:P])
                o3_ps = pp2.tile([P, 3 * H], f32, tag="ps2t", name="ps2d")
                nc.tensor.matmul(o3_ps[:, 0:1], lhsT=o2T[:], rhs=W3o[:], start=True, stop=False)
                nc.tensor.matmul(o3_ps[:, 0:1], lhsT=ones_row[:], rhs=b3o[:], start=False, stop=True)
                osig = sb.tile([P, 1], f32, tag="osig")
                nc.scalar.activation(osig[:], o3_ps[:, 0:1], ACT.Sigmoid)
                nc.sync.dma_start(out=out_t[:, :], in_=osig[:])
    nc.compile()
    # the BIR is frozen after compile; memoize its (deterministic) JSON
    # serialization so repeated run_bass_kernel_spmd calls skip re-encoding
    # ~5k instructions during jit lowering
    bir_json = nc.to_json_bytes()
    nc.to_json_bytes = lambda: bir_json
    return nc


def _run(nc, in_maps, trace=False):
    from concourse.bass_utils import run_bass_kernel_spmd

    res = run_bass_kernel_spmd(nc, in_maps, core_ids=list(range(NCORES)), trace=trace)
    return np.concatenate([res.results[c]["out"][:, 0] for c in range(NCORES)]), res


def kernel(x, edge_index, batch, enc_W, enc_b, gat_W, gat_a_src, gat_a_dst, gat_b,
           att_W1, att_b1, att_w2, att_b2, gru_Wih, gru_Whh, gru_bih, gru_bhh,
           out_W1, out_b1, out_W2, out_b2, out_W3, out_b3):
    x = np.asarray(x, np.float32)
    edge_index = np.asarray(edge_index)
    batch = np.asarray(batch).astype(np.int64)
    meta, offs, scat, xT = _build_host(x, edge_index, batch)
    nc = _build_bass(meta)
    weights = _pack_weights(enc_W, enc_b, gat_W, gat_a_src, gat_a_dst, gat_b,
                            att_W1, att_b1, att_w2, att_b2, gru_Wih, gru_Whh,
                            gru_bih, gru_bhh, out_W1, out_b1, out_W2, out_b2,
                            out_W3, out_b3)
    in_maps = _make_inmaps(meta, offs, scat, xT, weights)
    out, _ = _run(nc, in_maps)
    return out.astype(np.float32)
